# revision 20
# baseline (speedup 1.0000x reference)
"""Trainium2 kernel for nn_LinearRowShared4Bit: out = x @ W.T + bias where W is
dequantized from 4-bit packed weights with per-(16-row-group, 32-col-block)
fp16 norms.

8-core tensor-parallel over out_features (1024 rows/core). Per core:

  - View int32 packed weights (value = byte B in low 8 bits) as uint16 pairs
    [B, 0]; DMA-transpose quarter-shards [1024 o, 2048 cols] -> SBUF
    [128p, 16 chunk, 1024 o] (4KB-contiguous M2S reads -> ~278 GB/s). Byte
    k=64c+m of chunk c sits on partition p=2m -> (b,h)=(k//16,k%16) ->
    W.T rows i_lo=32b+2h (low nibble), i_lo+1 (high nibble); odd p are 0.
    All DMAs ride ONE HWDGE ring (nc.sync): concurrent plain DMAs corrupt
    in-flight xbar transposes (engine-global S2M xbar state).
  - Bit-assemble fp16 weights with pure-bitwise DVE ops (no int->fp convert):
      F_L = (T & 15) | 0x6400   == fp16(1024 + L)
      F_T =  T       | 0x6400   == fp16(1024 + T),  T = 16H + L
  - Stage 1 (PE): per chunk-pair, accumulate per-block-slot partials into
    PSUM [128=(16t x 8slot), 1024o] with host-prepped block-masked x patterns
    xepat (xe - xo/16) against F_L and xopat (xo/16) against F_T, so nibbles
    separate algebraically: sum x*s*q = sum(xe-xo/16)(s*L) + sum(xo/16)(s*T).
  - Stage 2: one fused DVE scalar_tensor_tensor per pair:
    (psum - K[m]) * s[m, og(o)], K = host-computed 1024-offset contribution;
    a selector matmul folds the 8 block-slots into PSUM out [16, 1024].
  - The "-norm" dequant term and bias ride a small fp32 side-matmul.

Host gathers per-core [16, 1024] outputs -> [16, 8192].
"""

import numpy as np

O, I = 8192, 8192
GROUP, SHARE = 32, 16
NCORES = 8
OS = O // NCORES          # 1024 out rows per core
OGS = OS // SHARE         # 64 row-groups per core
NCHUNK = I // 128         # 64 col-chunks of 128 uint16 columns
NPAIR = NCHUNK // 2
NQ = 4                    # quarter transposes, 16 chunks each
T_BATCH = 16

# packed f16 const layout (u16 columns): xep | xop | s2 | sel
_XEP0, _XOP0 = 0, NCHUNK * 64
_S20 = 2 * NCHUNK * 64
_SEL0 = _S20 + NPAIR * OGS
_C16W = _SEL0 + T_BATCH
# packed f32 const layout: koff | xsT | normT
_K0, _XST0, _NMT0 = 0, NPAIR, NPAIR + 2 * T_BATCH
_C32W = _NMT0 + 2 * OGS

_cache = {}


def _build_program():
    import concourse.mybir as mybir
    from concourse import bacc
    from concourse.tile import TileContext

    f16, f32, u16 = mybir.dt.float16, mybir.dt.float32, mybir.dt.uint16
    alu = mybir.AluOpType
    nc = bacc.Bacc("TRN2", target_bir_lowering=False, debug=False)

    wq16 = nc.dram_tensor("wq16", [OS, I], u16, kind="ExternalInput")
    c16_d = nc.dram_tensor("c16", [128, _C16W], u16, kind="ExternalInput")
    c32_d = nc.dram_tensor("c32", [128, _C32W], f32, kind="ExternalInput")
    fx_d = nc.dram_tensor("rhs_fix", [OGS + 1, OS], f32, kind="ExternalInput")
    out_d = nc.dram_tensor("out", [T_BATCH, OS], f32, kind="ExternalOutput")

    with TileContext(nc) as tc:
        with (
            tc.tile_pool(name="const", bufs=1) as const,
            tc.tile_pool(name="tp", bufs=2) as tp,
            tc.tile_pool(name="wp", bufs=3) as wp,
            tc.tile_pool(name="ps", bufs=1, space="PSUM") as ps,
        ):
            c16 = const.tile([128, _C16W], u16)
            nc.sync.dma_start(c16[:], c16_d[:])
            c32 = const.tile([128, _C32W], f32)
            nc.sync.dma_start(c32[:], c32_d[:])
            fx_sb = const.tile([OGS + 1, OS], f32)
            nc.sync.dma_start(fx_sb[:], fx_d[:])
            u32 = mybir.dt.uint32
            mask = const.tile([128, 1], u32)
            nc.vector.memset(mask[:], 0x000F000F)
            orc = const.tile([128, 1], u32)
            nc.vector.memset(orc[:], 0x64006400)

            xep_sb = c16[:, _XEP0:_XOP0].rearrange(
                "p (c m) -> p c m", m=64).bitcast(f16)
            xop_sb = c16[:, _XOP0:_S20].rearrange(
                "p (c m) -> p c m", m=64).bitcast(f16)
            s2_sb = c16[:, _S20:_SEL0].rearrange(
                "p (r g) -> p r g", g=OGS).bitcast(f16)
            sel_sb = c16[:, _SEL0:_C16W].bitcast(f16)
            k_sb = c32[:, _K0:_XST0]
            xsT_sb = c32[:, _XST0:_NMT0].rearrange("p (j t) -> p j t", t=T_BATCH)
            nmT_sb = c32[:, _NMT0:_C32W].rearrange("p (j g) -> p j g", g=OGS)

            psA = ps.tile([T_BATCH, 512], f32)
            psB = ps.tile([T_BATCH, 512], f32)

            # fix path: N.T[og, t] = sum_b norm[og, b] * xs[t, b]
            ps2 = ps.tile([OGS, T_BATCH], f32, tag="pp", bufs=3)
            nc.tensor.matmul(ps2[:], nmT_sb[:, 0, :], xsT_sb[:, 0, :],
                             start=True, stop=False)
            nc.tensor.matmul(ps2[:], nmT_sb[:, 1, :], xsT_sb[:, 1, :],
                             start=False, stop=True)
            fixw = const.tile([OGS + 1, T_BATCH], f32)
            nc.vector.tensor_scalar_mul(fixw[0:OGS, :], ps2[:], -1.0)
            nc.vector.memset(fixw[OGS:OGS + 1, :], 1.0)

            for q in range(NQ):
                TQ = tp.tile([128, 16, 1024], u16, tag="T", name=f"tq{q}")
                nc.sync.dma_start_transpose(
                    TQ[:], wq16[:, 2048 * q:2048 * (q + 1)])

                for pl in range(8):
                    pr = 8 * q + pl
                    T2 = TQ[:, 2 * pl:2 * pl + 2, :]

                    # host pre-ORs 0x6400 into every uint16, so T2 already IS
                    # F_T = fp16(1024 + T); only F_L needs assembling, done on
                    # a uint32 view (2 packed uint16 per element -> 2x mode)
                    FL = wp.tile([128, 2048], u16, tag="FL")
                    nc.vector.tensor_scalar(
                        FL[:].bitcast(u32), T2.bitcast(u32),
                        mask[:], orc[:], alu.bitwise_and, alu.bitwise_or)
                    FLh = FL[:].bitcast(f16)
                    FTh = TQ[:, 2 * pl:2 * pl + 2, :].rearrange(
                        "p a b -> p (a b)").bitcast(f16)

                    pp = ps.tile([128, 1024], f32, tag="pp", bufs=3)
                    for h in (0, 1):
                        c = 2 * pr + h
                        xe_l = xep_sb[:, c, :]
                        xo_l = xop_sb[:, c, :]
                        o0 = 1024 * h
                        rows = pp[64 * h:64 * h + 64, :]
                        nc.tensor.matmul(rows[:, 0:512], xe_l,
                                         FLh[:, o0:o0 + 512],
                                         start=True, stop=False)
                        nc.tensor.matmul(rows[:, 512:1024], xe_l,
                                         FLh[:, o0 + 512:o0 + 1024],
                                         start=True, stop=False)
                        nc.tensor.matmul(rows[:, 0:512], xo_l,
                                         FTh[:, o0:o0 + 512],
                                         start=False, stop=True)
                        nc.tensor.matmul(rows[:, 512:1024], xo_l,
                                         FTh[:, o0 + 512:o0 + 1024],
                                         start=False, stop=True)

                    sc = wp.tile([128, 1024], f16, tag="SC")
                    nc.vector.scalar_tensor_tensor(
                        sc[:].rearrange("p (a b) -> p a b", b=SHARE),
                        pp[:].rearrange("p (a b) -> p a b", b=SHARE),
                        k_sb[:, pr:pr + 1],
                        s2_sb[:, pr, :].unsqueeze(2).broadcast_to(
                            [128, OGS, SHARE]),
                        alu.subtract, alu.mult)

                    nc.tensor.matmul(psA[:], sel_sb[:], sc[:, 0:512],
                                     start=(pr == 0), stop=False)
                    nc.tensor.matmul(psB[:], sel_sb[:], sc[:, 512:1024],
                                     start=(pr == 0), stop=False)

            nc.tensor.matmul(psA[:], fixw[:], fx_sb[:, 0:512],
                             start=False, stop=True)
            nc.tensor.matmul(psB[:], fixw[:], fx_sb[:, 512:1024],
                             start=False, stop=True)
            out_sb = const.tile([T_BATCH, OS], f32)
            nc.vector.tensor_copy(out_sb[:, 0:512], psA[:])
            nc.vector.tensor_copy(out_sb[:, 512:1024], psB[:])
            nc.sync.dma_start(out_d[:], out_sb[:])

    nc.finalize()
    return nc


def _prep_shared(x):
    """x-derived operands, identical on every core."""
    xf = x.astype(np.float64)
    k = np.arange(I // 2)                   # byte index within a row
    b, h = k // 16, k % 16
    i_lo = 32 * b + 2 * h
    xe_mod = xf[:, i_lo] - xf[:, i_lo + 1] / 16.0   # [16, 4096]
    xo16 = xf[:, i_lo + 1] / 16.0                    # [16, 4096]

    def pat(a):
        """[16, 4096] -> [128, 64, 64] fp16 block-slot pattern.

        Payload lane p=2m of chunk c holds byte k=64c+m; its x value goes to
        column m' = 16*(p//32) + t. Odd lanes and other columns stay 0."""
        lane = np.zeros((128, NCHUNK, T_BATCH), np.float16)
        lane[0::2] = a.T.reshape(NCHUNK, 64, T_BATCH).transpose(1, 0, 2)
        out = np.zeros((128, NCHUNK, 64), np.float16)
        for jj in range(4):
            rows = slice(32 * jj, 32 * jj + 32)
            out[rows, :, 16 * jj:16 * jj + 16] = lane[rows]
        return out

    xep = pat(xe_mod)
    xop = pat(xo16)

    # K[m, pr] = 1024 * sum_p (xep + xop)[p, c, m%64] with c = 2pr + m//64,
    # computed from the fp16-rounded patterns (must match device exactly).
    colsum = (xep.astype(np.float64) + xop.astype(np.float64)).sum(axis=0)
    K = np.zeros((128, NPAIR), np.float32)
    K[0:64] = 1024.0 * colsum[0::2].T
    K[64:128] = 1024.0 * colsum[1::2].T

    sel = (np.arange(128)[:, None] % 16 == np.arange(T_BATCH)[None, :]
           ).astype(np.float16)

    xs = xf.reshape(T_BATCH, I // GROUP, GROUP).sum(-1)   # [16, 256]
    xsT = np.ascontiguousarray(
        xs.T.reshape(2, 128, T_BATCH).transpose(1, 0, 2)).astype(np.float32)
    return xep, xop, K, sel, xsT


def kernel(x, weight_q4, weight_norm, bias, _trace=False, _trace_kwargs=None):
    from concourse.bass_utils import run_bass_kernel_spmd

    if "nc" not in _cache:
        _cache["nc"] = _build_program()
    nc = _cache["nc"]

    xep, xop, K, sel, xsT = _prep_shared(x)
    selmat = (np.arange(OS) // SHARE == np.arange(OGS)[:, None]).astype(np.float32)

    c16 = np.empty((128, _C16W), np.uint16)
    c16[:, _XEP0:_XOP0] = xep.reshape(128, -1).view(np.uint16)
    c16[:, _XOP0:_S20] = xop.reshape(128, -1).view(np.uint16)
    c16[:, _SEL0:_C16W] = sel.view(np.uint16)

    c32 = np.empty((128, _C32W), np.float32)
    c32[:, _K0:_XST0] = K
    c32[:, _XST0:_NMT0] = xsT.reshape(128, -1)

    in_maps = []
    for m in range(NCORES):
        wq = np.ascontiguousarray(weight_q4[m * OS:(m + 1) * OS]).astype('<i4')
        # repack: OR the fp16 exponent 0x6400 into every uint16 half, so the
        # device-side transposed tiles directly read as fp16(1024 + byte)
        wq16 = (wq.view('<u2') | np.uint16(0x6400)).reshape(OS, I)

        norm = weight_norm[m * OGS:(m + 1) * OGS, :, 0].astype(np.float32)
        sn = (2.0 / 15.0) * norm
        # s2[m, pr, og] = sn[og, 8*pr + m//16]
        blk = 8 * np.arange(NPAIR)[None, :] + (np.arange(128) // 16)[:, None]
        s2 = sn.T[blk].astype(np.float16)                 # [128, 32, 64]

        nmT = np.ascontiguousarray(
            norm.T.reshape(2, 128, OGS).transpose(1, 0, 2)).astype(np.float32)

        c16m = c16.copy()
        c16m[:, _S20:_SEL0] = s2.reshape(128, -1).view(np.uint16)
        c32m = c32.copy()
        c32m[:, _NMT0:_C32W] = nmT.reshape(128, -1)

        rhs_fix = np.empty((OGS + 1, OS), np.float32)
        rhs_fix[0:OGS] = selmat
        rhs_fix[OGS] = bias[m * OS:(m + 1) * OS].astype(np.float32)

        in_maps.append(dict(wq16=wq16, c16=c16m, c32=c32m, rhs_fix=rhs_fix))

    res = run_bass_kernel_spmd(nc, in_maps, core_ids=list(range(NCORES)),
                               trace=_trace, **(_trace_kwargs or {}))
    outs = [r["out"] for r in res.results]
    full = np.concatenate(outs, axis=1).astype(np.float32)
    if _trace:
        return full, res
    return full
